# revision 19
# baseline (speedup 1.0000x reference)
"""Causal multi-head attention (b=2, s=2048, d=1024, h=16) on 8 TRN2 NeuronCores.

Sharding: DP=2 on batch x TP=4 on head groups (4 heads = 256 dims per core).
Host pre-transposes x and the weight slices so the device kernel is
transpose-free; the wo row-parallel partial sums + the bv/bo bias corrections
are applied on the host after gathering.

Device dataflow per core (matmuls in float32r for full PE rate):
  xT [1024,2048] -> QT/KT [256,2048] (bias added on VectorE), V [2048,4x65]
  (65th column = ones, used as the stationary operand of the softmax
  denominator matmuls).  Per head pair and sq chunk: scoresT [sk,sq] via
  row-tiled 2-head packed matmuls, exp on ScalarE (x1/8 folded into the
  activation scale), causal zeroing of the diag block on GpSimd post-exp,
  then column-tiled AV^T + denominator matmuls (both heads concurrently in
  disjoint PE column groups); softmax normalization via a batched DVE
  reciprocal + DRAM-bounce partition-broadcast.

  The attention phase is ScalarE(exp)-bound, so projection and wo matmuls
  are interleaved a few at a time between the scores and AV matmuls of every
  attention step -- the PE stream is issued in order, and this keeps it
  dense so the HAM activity monitor holds the PE clock at 2.4 GHz.
"""

import os

import numpy as np

D = 1024
S = 2048
B = 2
H = 16
DK = 64
TP = 4
DP = 2
EC = 256  # head dims per core
HPC = 4  # heads per core
NCORES = 8

TRACE = os.environ.get("KERNEL_TRACE", "0") == "1"
LAST_EXEC_NS = None

_compiled = {}


def _build_nc():
    import concourse.mybir as mybir
    from concourse import bacc, tile
    from concourse.bass import ts
    from itertools import chain

    f32 = mybir.dt.float32
    f32r = mybir.dt.float32r
    AF = mybir.ActivationFunctionType

    nc = bacc.Bacc("TRN2", target_bir_lowering=False, debug=False)

    xt_d = nc.dram_tensor("xt", [D, S], f32r, kind="ExternalInput").ap()
    wqt_d = nc.dram_tensor("wqt", [D, EC], f32r, kind="ExternalInput").ap()
    wkt_d = nc.dram_tensor("wkt", [D, EC], f32r, kind="ExternalInput").ap()
    wvt_d = nc.dram_tensor("wvt", [D, EC], f32r, kind="ExternalInput").ap()
    wot_d = nc.dram_tensor("wot", [EC, D], f32r, kind="ExternalInput").ap()
    bq_d = nc.dram_tensor("bq", [EC], f32, kind="ExternalInput").ap()
    bk_d = nc.dram_tensor("bk", [EC], f32, kind="ExternalInput").ap()
    out_d = nc.dram_tensor("out", [S, D], f32, kind="ExternalOutput").ap()

    KT = D // 128  # 8 contraction tiles
    NC_SQ = S // 512  # 4 sq chunks

    with tile.TileContext(nc) as tc:
        with (
            tc.tile_pool(name="persist", bufs=1) as persist,
            tc.tile_pool(name="work", bufs=1) as work,
            tc.tile_pool(name="psum", bufs=1, space="PSUM") as psum,
            tc.tile_pool(name="dram", bufs=2, space="DRAM") as dram,
        ):
            # ---- persistent SBUF tensors ----
            xt_sb = persist.tile([128, KT, S], f32r)  # x^T, d on partitions
            wqt_sb = persist.tile([128, KT, EC], f32r)
            wkt_sb = persist.tile([128, KT, EC], f32r)
            wvt_sb = persist.tile([128, KT, EC], f32r)
            wot_sb = persist.tile([128, 2, D], f32r)
            bq_sb = persist.tile([128, 2], f32)
            bk_sb = persist.tile([128, 2], f32)
            qt_sb = persist.tile([128, 2, S], f32r)  # head pairs stacked
            kt_sb = persist.tile([128, 2, S], f32r)
            v_sb = persist.tile([128, S // 128, HPC * (DK + 1)], f32r)
            avt_sb = persist.tile([128, 2, S], f32r)

            # ---- input DMAs, ordered so chunk-0 work can start ASAP ----
            xt_t = xt_d.rearrange("(k p) m -> k p m", p=128)
            for k in range(KT):
                nc.sync.dma_start(
                    out=wqt_sb[:, k, :],
                    in_=wqt_d.rearrange("(k p) m -> k p m", p=128)[k],
                )
                nc.sync.dma_start(
                    out=wkt_sb[:, k, :],
                    in_=wkt_d.rearrange("(k p) m -> k p m", p=128)[k],
                )
                nc.sync.dma_start(
                    out=xt_sb[:, k, ts(0, 512)], in_=xt_t[k][:, ts(0, 512)]
                )
                nc.sync.dma_start(
                    out=wvt_sb[:, k, :],
                    in_=wvt_d.rearrange("(k p) m -> k p m", p=128)[k],
                )
            nc.sync.dma_start(out=bq_sb, in_=bq_d.rearrange("(t p) -> p t", p=128))
            nc.sync.dma_start(out=bk_sb, in_=bk_d.rearrange("(t p) -> p t", p=128))
            for c in range(1, NC_SQ):
                for k in range(KT):
                    nc.sync.dma_start(
                        out=xt_sb[:, k, ts(c, 512)], in_=xt_t[k][:, ts(c, 512)]
                    )
            wot_t = wot_d.rearrange("(t p) m -> t p m", p=128)
            for t in range(2):
                nc.sync.dma_start(out=wot_sb[:, t, :], in_=wot_t[t])

            # ones column per head in V (stationary operand of the denom
            # matmuls); memset can't target f32r -> write through f32 view.
            v4 = v_sb.rearrange("p t (h e) -> p t h e", e=DK + 1)
            nc.vector.memset(v4[:, :, :, DK].bitcast(f32), 1.0)

            def qtkt_gen(c):
                """QT/KT projections for chunk c; yields once per matmul."""
                for dst_sb, w_sb, b_sb in (
                    (qt_sb, wqt_sb, bq_sb),
                    (kt_sb, wkt_sb, bk_sb),
                ):
                    for d2 in range(2):
                        ps = psum.tile([128, 512], f32, tag="proj", bufs=2)
                        for k in range(KT):
                            nc.tensor.matmul(
                                ps,
                                lhsT=w_sb[:, k, ts(d2, 128)],
                                rhs=xt_sb[:, k, ts(c, 512)],
                                start=(k == 0),
                                stop=(k == KT - 1),
                            )
                            if k == KT - 1:
                                nc.vector.tensor_scalar_add(
                                    out=dst_sb[:, d2, ts(c, 512)],
                                    in0=ps,
                                    scalar1=b_sb[:, d2 : d2 + 1],
                                )
                            yield

            def v_gen(tiles):
                """V projection for the given s-tiles; yields once per matmul."""
                for t in tiles:
                    ps = psum.tile([128, EC], f32, tag="proj", bufs=2)
                    for k in range(KT):
                        nc.tensor.matmul(
                            ps,
                            lhsT=xt_sb[:, k, ts(t, 128)],
                            rhs=wvt_sb[:, k, :],
                            start=(k == 0),
                            stop=(k == KT - 1),
                        )
                        if k == KT - 1:
                            nc.vector.tensor_copy(
                                out=v4[:, t, :, 0:DK],
                                in_=ps.rearrange("p (h e) -> p h e", e=DK),
                            )
                        yield

            def wo_gen(c):
                for t in range(4 * c, 4 * c + 4):
                    osb = work.tile([128, D], f32, tag="osb", bufs=2)
                    for n in range(2):
                        po = psum.tile([128, 512], f32, tag="proj", bufs=2)
                        for p2 in range(2):
                            nc.tensor.matmul(
                                po,
                                lhsT=avt_sb[:, p2, ts(t, 128)],
                                rhs=wot_sb[:, p2, ts(n, 512)],
                                start=(p2 == 0),
                                stop=(p2 == 1),
                            )
                            if p2 == 1:
                                nc.vector.tensor_copy(
                                    out=osb[:, ts(n, 512)], in_=po
                                )
                                if n == 1:
                                    nc.sync.dma_start(
                                        out=out_d[ts(t, 128), :], in_=osb
                                    )
                            yield

            def drain(gen, n=None):
                took = 0
                for _ in gen:
                    took += 1
                    if n is not None and took >= n:
                        break
                return took

            def attention_chunk(c, filler, quota):
                for pr in range(2):
                    hA, hB = 2 * pr, 2 * pr + 1
                    av = psum.tile([128, 512], f32, tag="av", bufs=2)
                    dn = psum.tile([65, 512], f32, tag="den", bufs=2)
                    # rows 1..31 of dn are never written by the denominator
                    # matmuls; preset them so the batched reciprocal below
                    # stays finite.
                    nc.vector.memset(dn, 1.0)
                    n_sk = 4 * c + 4
                    for i in range(n_sk):
                        off = max(0, 128 * i - 512 * c)
                        w = 512 - off
                        sq_lo = 512 * c + off
                        exp_tiles = []
                        for hh, (p_lo, p_hi) in enumerate(((0, 64), (64, 128))):
                            pscore = psum.tile([128, w], f32, tag="score", bufs=2)
                            nc.tensor.matmul(
                                pscore,
                                lhsT=kt_sb[p_lo:p_hi, pr, ts(i, 128)],
                                rhs=qt_sb[p_lo:p_hi, pr, sq_lo : sq_lo + w],
                                start=True,
                                stop=True,
                            )
                            et = work.tile([128, w], f32r, tag="exp", bufs=5)
                            nc.scalar.activation(
                                out=et, in_=pscore, func=AF.Exp, scale=0.125
                            )
                            if i >= 4 * c:
                                # causal: zero the lower triangle of the diag
                                # block post-exp on the otherwise-idle GpSimd
                                dv = et[:, 0:128]
                                nc.gpsimd.affine_select(
                                    out=dv,
                                    in_=dv,
                                    compare_op=mybir.AluOpType.is_ge,
                                    fill=0.0,
                                    base=0,
                                    pattern=[[1, 128]],
                                    channel_multiplier=-1,
                                )
                            exp_tiles.append(et)
                        # keep the in-order PE stream fed while the exps run
                        drain(filler, quota)
                        fl = (i == 0, i == n_sk - 1)
                        # AV^T for both heads in disjoint PE column groups
                        # (runs concurrently), then the two denominator rows.
                        nc.tensor.matmul(
                            av[0:64, off : off + w],
                            lhsT=v_sb[:, i, hA * 65 : hA * 65 + 64],
                            rhs=exp_tiles[0],
                            start=fl[0],
                            stop=fl[1],
                            skip_group_check=True,
                        )
                        nc.tensor.matmul(
                            av[64:128, off : off + w],
                            lhsT=v_sb[:, i, hB * 65 : hB * 65 + 64],
                            rhs=exp_tiles[1],
                            start=fl[0],
                            stop=fl[1],
                            skip_group_check=True,
                        )
                        nc.tensor.matmul(
                            dn[0:1, off : off + w],
                            lhsT=v_sb[:, i, hA * 65 + 64 : hA * 65 + 65],
                            rhs=exp_tiles[0],
                            start=fl[0],
                            stop=fl[1],
                            skip_group_check=True,
                        )
                        nc.tensor.matmul(
                            dn[64:65, off : off + w],
                            lhsT=v_sb[:, i, hB * 65 + 64 : hB * 65 + 65],
                            rhs=exp_tiles[1],
                            start=fl[0],
                            stop=fl[1],
                            skip_group_check=True,
                        )
                    # normalize the pair: batched reciprocal straight off the
                    # denominator psum rows (parked at partitions 0/32), then
                    # a DRAM round-trip to broadcast 1/denom across 64
                    # partitions (SBUF APs can't have a zero partition step;
                    # DRAM can).  No PE involvement.
                    rec = work.tile([128, 512], f32, tag="rec", bufs=2)
                    nc.vector.reciprocal(out=rec[0:65, :], in_=dn)
                    rec_dram = dram.tile([2, 512], f32, tag="rec_dram")
                    nc.sync.dma_start(
                        out=rec_dram,
                        in_=rec.rearrange("(a b) m -> a b m", b=64)[:, 0, :],
                    )
                    for hh in range(2):
                        recb = work.tile([64, 512], f32, tag="recb", bufs=2)
                        nc.sync.dma_start(
                            out=recb,
                            in_=rec_dram[hh : hh + 1, :].to_broadcast([64, 512]),
                        )
                        nc.vector.tensor_mul(
                            out=avt_sb[64 * hh : 64 * hh + 64, pr, ts(c, 512)],
                            in0=av[64 * hh : 64 * hh + 64, :],
                            in1=recb,
                        )

            # Filler plan (PE work interleaved into the ScalarE-bound
            # attention steps, paced so no chunk starves):
            #   prologue: QT/KT+V for chunk 0 (dense)
            #   chunk 0: QT/KT(1) + V tiles 4,5        (48 mms, 8 steps)
            #   chunk 1: V 6,7 + QT/KT(2) + V 8,9      (64 mms, 16 steps)
            #   chunk 2: V 10,11 + QT/KT(3) + V 12,13  (64 mms, 24 steps)
            #   chunk 3: V 14,15 + wo(0..2)            (64 mms, 32 steps)
            #   tail: wo(3)
            drain(qtkt_gen(0))
            drain(v_gen(range(0, 4)))
            plans = [
                (chain(qtkt_gen(1), v_gen([4, 5])), 6),
                (chain(v_gen([6, 7]), qtkt_gen(2), v_gen([8, 9])), 4),
                (chain(v_gen([10, 11]), qtkt_gen(3), v_gen([12, 13])), 3),
                (chain(v_gen([14, 15]), wo_gen(0), wo_gen(1), wo_gen(2)), 2),
            ]
            for c in range(NC_SQ):
                filler, quota = plans[c]
                attention_chunk(c, filler, quota)
                drain(filler)  # anything attention didn't absorb
            drain(wo_gen(NC_SQ - 1))

    nc.compile()
    return nc


def _get_nc():
    if "nc" not in _compiled:
        _compiled["nc"] = _build_nc()
    return _compiled["nc"]


def make_in_maps(x, wq, bq, wk, bk, wv, bv, wo, bo):
    """Host-side sharding: core c handles batch c//TP, heads 4*(c%TP)..+4."""
    in_maps = []
    for c in range(NCORES):
        b = c // TP
        hs = (c % TP) * EC
        he = hs + EC
        in_maps.append(
            {
                "xt": np.ascontiguousarray(x[b].T),
                "wqt": np.ascontiguousarray(wq[hs:he, :].T),
                "wkt": np.ascontiguousarray(wk[hs:he, :].T),
                "wvt": np.ascontiguousarray(wv[hs:he, :].T),
                "wot": np.ascontiguousarray(wo[:, hs:he].T),
                "bq": np.ascontiguousarray(bq[hs:he]),
                "bk": np.ascontiguousarray(bk[hs:he]),
            }
        )
    return in_maps


def combine_outputs(partials, wo, bv, bo):
    """Host-side unsharding: sum TP partials per batch + bias correction."""
    corr = (wo.astype(np.float32) @ bv.astype(np.float32)) + bo.astype(np.float32)
    out = np.zeros((B, S, D), dtype=np.float32)
    for b in range(B):
        acc = np.zeros((S, D), dtype=np.float32)
        for g in range(TP):
            acc += partials[b * TP + g]
        out[b] = acc + corr[None, :]
    return out


def kernel(x, wq, bq, wk, bk, wv, bv, wo, bo):
    global LAST_EXEC_NS
    from concourse.bass_utils import run_bass_kernel_spmd

    x = np.asarray(x, dtype=np.float32)
    wq = np.asarray(wq, dtype=np.float32)
    bq = np.asarray(bq, dtype=np.float32)
    wk = np.asarray(wk, dtype=np.float32)
    bk = np.asarray(bk, dtype=np.float32)
    wv = np.asarray(wv, dtype=np.float32)
    bv = np.asarray(bv, dtype=np.float32)
    wo = np.asarray(wo, dtype=np.float32)
    bo = np.asarray(bo, dtype=np.float32)

    nc = _get_nc()
    in_maps = make_in_maps(x, wq, bq, wk, bk, wv, bv, wo, bo)
    res = run_bass_kernel_spmd(
        nc, in_maps, core_ids=list(range(NCORES)), trace=TRACE
    )
    LAST_EXEC_NS = res.exec_time_ns
    _compiled["last_res"] = res
    partials = [res.results[c]["out"] for c in range(NCORES)]
    return combine_outputs(partials, wo, bv, bo)


# revision 20
# speedup vs baseline: 1.1233x; 1.1233x over previous
"""Causal multi-head attention (b=2, s=2048, d=1024, h=16) on 8 TRN2 NeuronCores.

Sharding: DP=2 on batch x TP=4 on head groups (4 heads = 256 dims per core).
Host pre-transposes x and the weight slices so the device kernel is
transpose-free; the wo row-parallel partial sums + the bv/bo bias corrections
are applied on the host after gathering.

Device dataflow per core (matmuls in float32r for full PE rate):
  xT [1024,2048] -> QT/KT [256,2048] (bias added on VectorE), V [2048,4x65]
  (65th column = ones, used as the stationary operand of the softmax
  denominator matmuls).  Per head pair and sq chunk: scoresT [sk,sq] via
  row-tiled 2-head packed matmuls, exp on ScalarE (x1/8 folded into the
  activation scale), causal zeroing of the diag block on GpSimd post-exp,
  then column-tiled AV^T + denominator matmuls (both heads concurrently in
  disjoint PE column groups); softmax normalization via a batched DVE
  reciprocal + DRAM-bounce partition-broadcast.

  The attention phase is ScalarE(exp)-bound, so projection and wo matmuls
  are interleaved a few at a time between the scores and AV matmuls of every
  attention step -- the PE stream is issued in order, and this keeps it
  dense so the HAM activity monitor holds the PE clock at 2.4 GHz.
"""

import os

import numpy as np

D = 1024
S = 2048
B = 2
H = 16
DK = 64
TP = 4
DP = 2
EC = 256  # head dims per core
HPC = 4  # heads per core
NCORES = 8

TRACE = os.environ.get("KERNEL_TRACE", "0") == "1"
LAST_EXEC_NS = None

_compiled = {}


def _build_nc():
    import concourse.mybir as mybir
    from concourse import bacc, tile
    from concourse.bass import ts
    from itertools import chain

    f32 = mybir.dt.float32
    f32r = mybir.dt.float32r
    AF = mybir.ActivationFunctionType

    nc = bacc.Bacc("TRN2", target_bir_lowering=False, debug=False)

    xt_d = nc.dram_tensor("xt", [D, S], f32r, kind="ExternalInput").ap()
    wqt_d = nc.dram_tensor("wqt", [D, EC], f32r, kind="ExternalInput").ap()
    wkt_d = nc.dram_tensor("wkt", [D, EC], f32r, kind="ExternalInput").ap()
    wvt_d = nc.dram_tensor("wvt", [D, EC], f32r, kind="ExternalInput").ap()
    wot_d = nc.dram_tensor("wot", [EC, D], f32r, kind="ExternalInput").ap()
    bq_d = nc.dram_tensor("bq", [EC], f32, kind="ExternalInput").ap()
    bk_d = nc.dram_tensor("bk", [EC], f32, kind="ExternalInput").ap()
    out_d = nc.dram_tensor("out", [S, D], f32, kind="ExternalOutput").ap()

    KT = D // 128  # 8 contraction tiles
    NC_SQ = S // 512  # 4 sq chunks

    with tile.TileContext(nc) as tc:
        with (
            tc.tile_pool(name="persist", bufs=1) as persist,
            tc.tile_pool(name="work", bufs=1) as work,
            tc.tile_pool(name="psum", bufs=1, space="PSUM") as psum,
            tc.tile_pool(name="dram", bufs=2, space="DRAM") as dram,
        ):
            # ---- persistent SBUF tensors ----
            xt_sb = persist.tile([128, KT, S], f32r)  # x^T, d on partitions
            wqt_sb = persist.tile([128, KT, EC], f32r)
            wkt_sb = persist.tile([128, KT, EC], f32r)
            wvt_sb = persist.tile([128, KT, EC], f32r)
            wot_sb = persist.tile([128, 2, D], f32r)
            bq_sb = persist.tile([128, 2], f32)
            bk_sb = persist.tile([128, 2], f32)
            qt_sb = persist.tile([128, 2, S], f32r)  # head pairs stacked
            kt_sb = persist.tile([128, 2, S], f32r)
            v_sb = persist.tile([128, S // 128, HPC * (DK + 1)], f32r)
            avt_sb = persist.tile([128, 2, S], f32r)

            # ---- input DMAs, ordered so chunk-0 work can start ASAP ----
            xt_t = xt_d.rearrange("(k p) m -> k p m", p=128)
            for k in range(KT):
                nc.sync.dma_start(
                    out=wqt_sb[:, k, :],
                    in_=wqt_d.rearrange("(k p) m -> k p m", p=128)[k],
                )
                nc.sync.dma_start(
                    out=wkt_sb[:, k, :],
                    in_=wkt_d.rearrange("(k p) m -> k p m", p=128)[k],
                )
                nc.sync.dma_start(
                    out=xt_sb[:, k, ts(0, 512)], in_=xt_t[k][:, ts(0, 512)]
                )
                nc.sync.dma_start(
                    out=wvt_sb[:, k, :],
                    in_=wvt_d.rearrange("(k p) m -> k p m", p=128)[k],
                )
            nc.sync.dma_start(out=bq_sb, in_=bq_d.rearrange("(t p) -> p t", p=128))
            nc.sync.dma_start(out=bk_sb, in_=bk_d.rearrange("(t p) -> p t", p=128))
            for c in range(1, NC_SQ):
                for k in range(KT):
                    nc.sync.dma_start(
                        out=xt_sb[:, k, ts(c, 512)], in_=xt_t[k][:, ts(c, 512)]
                    )
            wot_t = wot_d.rearrange("(t p) m -> t p m", p=128)
            for t in range(2):
                nc.sync.dma_start(out=wot_sb[:, t, :], in_=wot_t[t])

            # ones column per head in V (stationary operand of the denom
            # matmuls); memset can't target f32r -> write through f32 view.
            v4 = v_sb.rearrange("p t (h e) -> p t h e", e=DK + 1)
            nc.vector.memset(v4[:, :, :, DK].bitcast(f32), 1.0)

            def qtkt_gen(c):
                """QT/KT projections for chunk c; yields once per matmul."""
                for dst_sb, w_sb, b_sb in (
                    (qt_sb, wqt_sb, bq_sb),
                    (kt_sb, wkt_sb, bk_sb),
                ):
                    for d2 in range(2):
                        ps = psum.tile([128, 512], f32, tag="proj", bufs=2)
                        for k in range(KT):
                            nc.tensor.matmul(
                                ps,
                                lhsT=w_sb[:, k, ts(d2, 128)],
                                rhs=xt_sb[:, k, ts(c, 512)],
                                start=(k == 0),
                                stop=(k == KT - 1),
                            )
                            if k == KT - 1:
                                nc.vector.tensor_scalar_add(
                                    out=dst_sb[:, d2, ts(c, 512)],
                                    in0=ps,
                                    scalar1=b_sb[:, d2 : d2 + 1],
                                )
                            yield

            def v_gen(tiles):
                """V projection for the given s-tiles; yields once per matmul."""
                for t in tiles:
                    ps = psum.tile([128, EC], f32, tag="proj", bufs=2)
                    for k in range(KT):
                        nc.tensor.matmul(
                            ps,
                            lhsT=xt_sb[:, k, ts(t, 128)],
                            rhs=wvt_sb[:, k, :],
                            start=(k == 0),
                            stop=(k == KT - 1),
                        )
                        if k == KT - 1:
                            nc.vector.tensor_copy(
                                out=v4[:, t, :, 0:DK],
                                in_=ps.rearrange("p (h e) -> p h e", e=DK),
                            )
                        yield

            def wo_gen(c):
                for t in range(4 * c, 4 * c + 4):
                    osb = work.tile([128, D], f32, tag="osb", bufs=2)
                    for n in range(2):
                        po = psum.tile([128, 512], f32, tag="proj", bufs=2)
                        for p2 in range(2):
                            nc.tensor.matmul(
                                po,
                                lhsT=avt_sb[:, p2, ts(t, 128)],
                                rhs=wot_sb[:, p2, ts(n, 512)],
                                start=(p2 == 0),
                                stop=(p2 == 1),
                            )
                            if p2 == 1:
                                nc.vector.tensor_copy(
                                    out=osb[:, ts(n, 512)], in_=po
                                )
                                if n == 1:
                                    nc.sync.dma_start(
                                        out=out_d[ts(t, 128), :], in_=osb
                                    )
                            yield

            def drain(gen, n=None):
                took = 0
                for _ in gen:
                    took += 1
                    if n is not None and took >= n:
                        break
                return took

            def attention_chunk(c, filler, quota):
                for pr in range(2):
                    hA, hB = 2 * pr, 2 * pr + 1
                    pa = psum.tile([65, 512], f32, tag="av", bufs=4)
                    pb = psum.tile([65, 512], f32, tag="av", bufs=4)
                    n_sk = 4 * c + 4
                    for i in range(n_sk):
                        off = max(0, 128 * i - 512 * c)
                        w = 512 - off
                        sq_lo = 512 * c + off
                        exp_tiles = []
                        for hh, (p_lo, p_hi) in enumerate(((0, 64), (64, 128))):
                            pscore = psum.tile([128, w], f32, tag="score", bufs=2)
                            nc.tensor.matmul(
                                pscore,
                                lhsT=kt_sb[p_lo:p_hi, pr, ts(i, 128)],
                                rhs=qt_sb[p_lo:p_hi, pr, sq_lo : sq_lo + w],
                                start=True,
                                stop=True,
                            )
                            et = work.tile([128, w], f32r, tag="exp", bufs=5)
                            nc.scalar.activation(
                                out=et, in_=pscore, func=AF.Exp, scale=0.125
                            )
                            if i >= 4 * c:
                                # causal: zero the lower triangle of the diag
                                # block post-exp on the otherwise-idle GpSimd
                                dv = et[:, 0:128]
                                nc.gpsimd.affine_select(
                                    out=dv,
                                    in_=dv,
                                    compare_op=mybir.AluOpType.is_ge,
                                    fill=0.0,
                                    base=0,
                                    pattern=[[1, 128]],
                                    channel_multiplier=-1,
                                )
                            exp_tiles.append(et)
                        # keep the in-order PE stream fed while the exps run
                        drain(filler, quota)
                        for hh, p_av in enumerate((pa, pb)):
                            h = 2 * pr + hh
                            nc.tensor.matmul(
                                p_av[:, off : off + w],
                                lhsT=v_sb[:, i, h * 65 : h * 65 + 65],
                                rhs=exp_tiles[hh],
                                start=(i == 0),
                                stop=(i == n_sk - 1),
                                skip_group_check=True,
                            )
                    # normalize the pair: batched reciprocal (denoms parked at
                    # partitions 0/32 -- engine APs must start 32-aligned),
                    # then a DRAM round-trip to broadcast 1/denom across 64
                    # partitions (SBUF APs can't have a zero partition step;
                    # DRAM can).  No PE involvement.
                    den = work.tile([64, 512], f32, tag="den", bufs=1)
                    nc.vector.memset(den, 1.0)
                    nc.vector.tensor_copy(out=den[0:1, :], in_=pa[64:65, :])
                    nc.vector.tensor_copy(out=den[32:33, :], in_=pb[64:65, :])
                    rec = work.tile([64, 512], f32, tag="rec", bufs=2)
                    nc.vector.reciprocal(out=rec, in_=den)
                    rec_dram = dram.tile([2, 512], f32, tag="rec_dram")
                    nc.sync.dma_start(
                        out=rec_dram,
                        in_=rec.rearrange("(a b) m -> a b m", b=32)[:, 0, :],
                    )
                    for hh, p_av in enumerate((pa, pb)):
                        recb = work.tile([64, 512], f32, tag="recb", bufs=2)
                        nc.sync.dma_start(
                            out=recb,
                            in_=rec_dram[hh : hh + 1, :].to_broadcast([64, 512]),
                        )
                        nc.vector.tensor_mul(
                            out=avt_sb[64 * hh : 64 * hh + 64, pr, ts(c, 512)],
                            in0=p_av[0:64, :],
                            in1=recb,
                        )

            # Filler plan (PE work interleaved into the ScalarE-bound
            # attention steps, paced so no chunk starves):
            #   prologue: QT/KT+V for chunk 0 (dense)
            #   chunk 0: QT/KT(1) + V tiles 4,5        (48 mms, 8 steps)
            #   chunk 1: V 6,7 + QT/KT(2) + V 8,9      (64 mms, 16 steps)
            #   chunk 2: V 10,11 + QT/KT(3) + V 12,13  (64 mms, 24 steps)
            #   chunk 3: V 14,15 + wo(0..2)            (64 mms, 32 steps)
            #   tail: wo(3)
            drain(qtkt_gen(0))
            drain(v_gen(range(0, 4)))
            plans = [
                (chain(qtkt_gen(1), v_gen([4, 5])), 6),
                (chain(v_gen([6, 7]), qtkt_gen(2), v_gen([8, 9])), 4),
                (chain(v_gen([10, 11]), qtkt_gen(3), v_gen([12, 13])), 3),
                (chain(v_gen([14, 15]), wo_gen(0), wo_gen(1), wo_gen(2)), 2),
            ]
            for c in range(NC_SQ):
                filler, quota = plans[c]
                attention_chunk(c, filler, quota)
                drain(filler)  # anything attention didn't absorb
            drain(wo_gen(NC_SQ - 1))

    nc.compile()
    return nc


def _get_nc():
    if "nc" not in _compiled:
        _compiled["nc"] = _build_nc()
    return _compiled["nc"]


def make_in_maps(x, wq, bq, wk, bk, wv, bv, wo, bo):
    """Host-side sharding: core c handles batch c//TP, heads 4*(c%TP)..+4."""
    in_maps = []
    for c in range(NCORES):
        b = c // TP
        hs = (c % TP) * EC
        he = hs + EC
        in_maps.append(
            {
                "xt": np.ascontiguousarray(x[b].T),
                "wqt": np.ascontiguousarray(wq[hs:he, :].T),
                "wkt": np.ascontiguousarray(wk[hs:he, :].T),
                "wvt": np.ascontiguousarray(wv[hs:he, :].T),
                "wot": np.ascontiguousarray(wo[:, hs:he].T),
                "bq": np.ascontiguousarray(bq[hs:he]),
                "bk": np.ascontiguousarray(bk[hs:he]),
            }
        )
    return in_maps


def combine_outputs(partials, wo, bv, bo):
    """Host-side unsharding: sum TP partials per batch + bias correction."""
    corr = (wo.astype(np.float32) @ bv.astype(np.float32)) + bo.astype(np.float32)
    out = np.zeros((B, S, D), dtype=np.float32)
    for b in range(B):
        acc = np.zeros((S, D), dtype=np.float32)
        for g in range(TP):
            acc += partials[b * TP + g]
        out[b] = acc + corr[None, :]
    return out


def kernel(x, wq, bq, wk, bk, wv, bv, wo, bo):
    global LAST_EXEC_NS
    from concourse.bass_utils import run_bass_kernel_spmd

    x = np.asarray(x, dtype=np.float32)
    wq = np.asarray(wq, dtype=np.float32)
    bq = np.asarray(bq, dtype=np.float32)
    wk = np.asarray(wk, dtype=np.float32)
    bk = np.asarray(bk, dtype=np.float32)
    wv = np.asarray(wv, dtype=np.float32)
    bv = np.asarray(bv, dtype=np.float32)
    wo = np.asarray(wo, dtype=np.float32)
    bo = np.asarray(bo, dtype=np.float32)

    nc = _get_nc()
    in_maps = make_in_maps(x, wq, bq, wk, bk, wv, bv, wo, bo)
    res = run_bass_kernel_spmd(
        nc, in_maps, core_ids=list(range(NCORES)), trace=TRACE
    )
    LAST_EXEC_NS = res.exec_time_ns
    _compiled["last_res"] = res
    partials = [res.results[c]["out"] for c in range(NCORES)]
    return combine_outputs(partials, wo, bv, bo)
